# revision 47
# baseline (speedup 1.0000x reference)
"""Trainium2 Bass kernel for 3-round GNN message passing + GRU encoder (GGM).

Math (per round k):
    msg_e = h[dst_e] @ Wv + h[src_e] @ Ww + e_e @ We + cond_row_k      [E, 128]
    m_v   = sum_{e: dst_e = v} msg_e                                  [N, 128]
    h_v   = GRU_k(m_v, h_v)
Since segment-sum is linear this is restructured as
    m_v = (deg_v * h_v) @ Wv + S_v @ Ww + EA_v @ We + deg_v * cond_row_k
with S_v = sum over in-edges of h[src] (the only gather-heavy term) and
EA_v = sum over in-edges of e = (segment_sum(edge_feats, dst)) @ W_edge_emb
(round-independent; accumulated on device during round 0's selection matmuls).

Sharding: nodes are split into 8 contiguous shards; each core owns the
in-edges of its shard (dst-sorted, in 128-edge tiles).  The gather table
(full h, node-major) is replicated in HBM and rebuilt each round with an
8-core AllGather.  Per-tile segment sums: a selection matrix Sel[e, d] =
(dstloc[e] == d) built by one is_equal op turns the segment-sum into a
PSUM-accumulated matmul  S^T[:, blk] += G_tile^T-style  (out = lhsT.T @ rhs
with lhsT = gathered h rows, rhs = Sel).  All node compute is feature-major
([128 feat] x nodes) so weights are natural lhsT operands and biases fuse
into ScalarE activations.  Readout: local free-dim reduction + AllReduce.
"""

import os
import sys

import numpy as np

for _p in ("/opt/trn_rl_repo",):
    if os.path.isdir(_p) and _p not in sys.path:
        sys.path.insert(0, _p)

import concourse.bacc as bacc
import concourse.bass as bass
import concourse.bass_isa as bass_isa
import concourse.mybir as mybir
import concourse.tile as tile
from concourse import tile_sem_assignment as _tsa
from concourse.masks import make_identity


def _install_queue_aware_dma_lanes():
    """Make Tile's SWDGE completion-sem lane choice queue-consistent.

    Tile round-robins Pool-engine DMA instructions over the 8 DMASW sem
    lanes with no knowledge of the SWDGE queue they run on, but a sem lane
    must only ever be updated from one SWDGE queue (ucode reclaim tracks
    per-queue).  Derive the lane from the instruction's queue instead:
    queue q uses lanes {2q, 2q+1}.
    """
    if getattr(_tsa.TileClockTick, "_ggm_patched", False):
        return
    orig = _tsa.TileClockTick._assign_tick

    def patched(self, inst):
        if (isinstance(inst, _tsa.DMAInst)
                and inst.engine == mybir.EngineType.Pool
                and not isinstance(inst, bass_isa.UserSyncedRemoteDMADescs)):
            q = int(getattr(inst, "queue_num", 0) or 0)
            cnts = getattr(self, "_ggm_qcnt", None)
            if cnts is None:
                cnts = self._ggm_qcnt = [0, 0, 0, 0]
            self.next_sw_dma_idx = 2 * q + (cnts[q] & 1)
            cnts[q] += 1
        return orig(self, inst)

    _tsa.TileClockTick._assign_tick = patched
    _tsa.TileClockTick._ggm_patched = True

FP = mybir.dt.float32
BF = mybir.dt.bfloat16
I16 = mybir.dt.int16

P = 128
DNV = 128
NA_IN = 14
NB_IN = 11
NCOND = 4
NROUNDS = 3

N_FULL = 50000
NCORES = 8

SBW = 4          # dst blocks per super-block (matmul free dim up to 512)
GCAP = 8         # max edge tiles per dma_gather: 1024 idxs = 1024 descs,
                 # the SWDGE ring capacity (16KB / 16B) — more wedges the HW
DST_SENTINEL = 300.0


# --------------------------------------------------------------------------
# host-side planning (integer ops / permutations only)
# --------------------------------------------------------------------------

class Plan:
    pass


def build_plan(edge_src, edge_dst, edge_feats, node_feats, n, ncores):
    """Shard + dst-sort edges; pad to a tile structure common to all cores."""
    pl = Plan()
    assert n % ncores == 0
    nsh = n // ncores
    assert nsh % P != 0 or True
    nblk = -(-nsh // P)
    nsb = -(-nblk // SBW)
    half = (n + 1) // 2
    pl.n, pl.nsh, pl.nblk, pl.nsb, pl.half = n, nsh, nblk, nsb, half
    pl.nsh_pad = nblk * P

    assert nblk >= 2
    ch1 = (nblk // 2) * P          # chunk-1 nodes per core (block-aligned)
    ch2 = nsh - ch1
    assert ncores * ch1 < 32768 and ncores * ch2 < 32768
    pl.ch1, pl.ch2 = ch1, ch2
    core_of = edge_dst // nsh
    per_core = []
    cnt = np.zeros((ncores, nblk, 2), np.int64)
    for c in range(ncores):
        sel = np.nonzero(core_of == c)[0]
        d = edge_dst[sel] - c * nsh
        s = edge_src[sel]
        s_core = s // nsh
        s_loc = s - s_core * nsh
        hlf = (s_loc >= ch1).astype(np.int64)
        blk = d // P
        order = np.lexsort((d, blk, hlf, blk // SBW))
        gval = np.where(hlf == 0, s_core * ch1 + s_loc,
                        s_core * ch2 + (s_loc - ch1)).astype(np.int16)
        sel, d, gval, hlf, blk = (sel[order], d[order], gval[order],
                                  hlf[order], blk[order])
        per_core.append((sel, d, gval, hlf, blk))
        np.add.at(cnt[c], (blk, hlf), 1)

    # common per-(block,half) tile counts: max over cores, at least 1
    ntile = np.maximum(1, -(-cnt.max(axis=0) // P))   # [nblk, 2]
    pl.ntile = ntile

    # ordered tile list (per sblock: half A blocks asc, then half B) and
    # gather groups (runs of tiles with one src half, <= gmax tiles)
    tiles = []    # (blk, half, start_flag, stop_flag)
    runs = []     # (half, tile_lo, tile_hi) one per (sblock, half)
    sb_span = []  # (tile_lo, tile_hi) per sblock
    trange = {}   # (blk, half) -> (tile_lo, tile_hi)
    for sb in range(nsb):
        blks = range(sb * SBW, min((sb + 1) * SBW, nblk))
        t_sb0 = len(tiles)
        for h in (0, 1):
            glo = len(tiles)
            for b in blks:
                trange[(b, h)] = (len(tiles), len(tiles) + ntile[b, h])
                for t in range(ntile[b, h]):
                    tiles.append((b, h, h == 0 and t == 0,
                                  h == 1 and t == ntile[b, h] - 1))
            runs.append((h, glo, len(tiles)))
        sb_span.append((t_sb0, len(tiles)))
    T = len(tiles)
    gmax = min(GCAP, max(hi - lo for _h, lo, hi in runs))
    groups = []   # (half, tile_lo, tile_hi) gather instructions
    for h, glo, ghi in runs:
        lo = glo
        while lo < ghi:
            hi = min(lo + gmax, ghi)
            groups.append((h, lo, hi))
            lo = hi
    pl.tiles, pl.groups, pl.T, pl.sb_span = tiles, groups, T, sb_span
    pl.trange, pl.gmax = trange, gmax
    # all of one super-block's gather groups are live at once (each block's
    # PSUM accumulation reads from both halves' groups) — size the pool for
    # that plus one slot of next-super-block prefetch
    per_sb = []
    gi = 0
    for lo_sb, hi_sb in sb_span:
        cnt = 0
        while gi < len(groups) and groups[gi][1] < hi_sb:
            cnt += 1
            gi += 1
        per_sb.append(cnt)
    pl.gbufs = max(per_sb) + 1

    # base slot of each (blk, half) run in the padded edge stream
    base = {}
    slot = 0
    for (b, h, _st, _sp) in tiles:
        if (b, h) not in base:
            base[(b, h)] = slot
        slot += P

    NC2 = 32 + NA_IN   # ef at cols 0..11, nf at cols 32..46 (matmul base-partition rule)
    pl.gidx = np.zeros((ncores, P, T * 8), np.int16)
    pl.dstloc = np.full((ncores, P, T), DST_SENTINEL, np.float32)
    pl.ef = np.zeros((ncores, P, T * NC2), np.float32)
    pl.degb = np.zeros((ncores, P, pl.nsh_pad), np.float32)
    pl.nfT = np.zeros((ncores, NA_IN, pl.nsh_pad), np.float32)

    deg_full = np.bincount(edge_dst, minlength=n).astype(np.float32)
    nfT_full = np.ascontiguousarray(node_feats.T.astype(np.float32))

    for c in range(ncores):
        sel, d, gval, hlf, blk = per_core[c]
        ne = len(sel)
        posn = np.zeros(ne, np.int64)
        if ne:
            key = blk * 2 + hlf
            bounds = np.nonzero(np.diff(key) != 0)[0] + 1
            starts = np.concatenate(([0], bounds))
            ends = np.concatenate((bounds, [ne]))
            for a, b_ in zip(starts, ends):
                posn[a:b_] = np.arange(b_ - a)
        slots = (np.array([base[(bb, hh)] for bb, hh in zip(blk, hlf)],
                          np.int64) + posn)
        gidx = gval
        col = (slots // 128) * 8 + (slots % 128) // 16
        row = slots % 16
        for r in range(8):
            pl.gidx[c, row + 16 * r, col] = gidx
        pl.dstloc[c, slots % P, slots // P] = (d - blk * P).astype(np.float32)
        ef_rows = edge_feats[sel].astype(np.float32)
        nf_rows = node_feats[edge_src[sel]].astype(np.float32)  # h0[src]
        tidx, pp = slots // P, slots % P
        for j in range(NB_IN):
            pl.ef[c, pp, tidx * NC2 + j] = ef_rows[:, j]
        for j in range(NA_IN):
            pl.ef[c, pp, tidx * NC2 + 32 + j] = nf_rows[:, j]
        pl.degb[c, :, :nsh] = np.broadcast_to(
            deg_full[c * nsh:(c + 1) * nsh], (P, nsh))
        pl.nfT[c, :, :nsh] = nfT_full[:, c * nsh:(c + 1) * nsh]

    return pl


# --------------------------------------------------------------------------
# device program
# --------------------------------------------------------------------------

def build_program(pl, ncores):
    _install_queue_aware_dma_lanes()
    nc = bacc.Bacc("TRN2", target_bir_lowering=False, debug=False,
                   num_devices=ncores, num_swdge_queues=4)
    n, nsh, nblk, nsb, T = pl.n, pl.nsh, pl.nblk, pl.nsb, pl.T
    half = pl.half
    nsh_pad = pl.nsh_pad
    inv_n = 1.0 / float(n)
    AF = mybir.ActivationFunctionType
    OP = mybir.AluOpType

    def din(name, shape, dtype=FP):
        return nc.dram_tensor(name, list(shape), dtype, kind="ExternalInput")

    gidx_d = din("gidx", (P, T * 8), I16)
    dstloc_d = din("dstloc", (P, T))
    NC2 = 32 + NA_IN
    ef_d = din("ef", (P, T * NC2))
    degb_d = din("degb", (P, nsh_pad))
    nfT_d = din("nfT", (NA_IN, nsh_pad))
    iota_d = din("iota", (P, P))
    wne_d = din("wne", (NA_IN, DNV))
    wee_d = din("wee", (NB_IN, DNV))
    uw_d = din("uw", (NROUNDS, 2 * DNV + DNV + NCOND, DNV))
    ub_d = din("ub", (NROUNDS, DNV, 1))
    wih_d = din("wih", (NROUNDS, DNV, 3 * DNV))
    whh_d = din("whh", (NROUNDS, DNV, 3 * DNV))
    bih_d = din("bih", (NROUNDS, DNV, 3))
    bhh_d = din("bhh", (NROUNDS, DNV, 3))
    condT_d = din("condT", (NCOND, 1))
    epsT_d = din("epsT", (DNV, 1))
    c1w_d = din("c1w", (DNV, DNV))
    c1b_d = din("c1b", (DNV, 1))
    c2w_d = din("c2w", (DNV, DNV))
    c2b_d = din("c2b", (DNV, 1))
    mw_d = din("mw", (DNV, DNV))
    mb_d = din("mb", (DNV, 1))
    lw_d = din("lw", (DNV, DNV))
    lb_d = din("lb", (DNV, 1))

    out3 = nc.dram_tensor("out3", [P, 3], FP, kind="ExternalOutput")

    rg = [list(range(ncores))]

    with tile.TileContext(nc) as tc:
        with (
            tc.tile_pool(name="cst", bufs=1) as cst,
            tc.tile_pool(name="dram", bufs=1, space="DRAM") as dram,
            tc.tile_pool(name="gpool", bufs=pl.gbufs) as gpool,
            tc.tile_pool(name="selp", bufs=pl.gbufs) as selp,
            tc.tile_pool(name="epool", bufs=2) as epool,
            tc.tile_pool(name="dpool", bufs=2) as dpool,
            tc.tile_pool(name="work", bufs=2) as work,
            tc.tile_pool(name="spsum", bufs=2, space="PSUM") as spsum,
            tc.tile_pool(name="mpsum", bufs=2, space="PSUM") as mpsum,
            tc.tile_pool(name="gpsum", bufs=4, space="PSUM") as gpsum,
        ):
            _regs = {}

            def nreg(v):
                if v not in _regs:
                    _regs[v] = nc.gpsimd.to_reg(v)
                return _regs[v]

            def load(dr_ap, shape, dtype=FP, name="w"):
                t = cst.tile(list(shape), dtype, name=name)
                nc.sync.dma_start(out=t[:], in_=dr_ap)
                return t

            def loadbf(dr_ap, shape, name="w"):
                t = cst.tile(list(shape), BF, name=name)
                nc.gpsimd.dma_start(out=t[:], in_=dr_ap)  # f32 -> bf16 cast
                return t

            idx_s = load(gidx_d[:], (P, T * 8), I16, "idx_s")
            dst_s = cst.tile([P, T], BF, name="dst_s")
            nc.gpsimd.dma_start(out=dst_s[:], in_=dstloc_d[:])  # f32->bf16
            iota_s = cst.tile([P, P], BF, name="iota_s")
            nc.gpsimd.dma_start(out=iota_s[:], in_=iota_d[:])
            wne_s = load(wne_d[:], (NA_IN, DNV), FP, "wne_s")
            wnebf_s = cst.tile([32 + NA_IN, DNV], BF, name="wnebf_s")
            nc.gpsimd.dma_start(out=wnebf_s[32:32 + NA_IN, :], in_=wne_d[:])
            wee_s = loadbf(wee_d[:], (NB_IN, DNV), "wee_s")
            condT_s = load(condT_d[:], (NCOND, 1), FP, "condT_s")
            epsT_s = load(epsT_d[:], (DNV, 1), FP, "epsT_s")
            c1w_s = loadbf(c1w_d[:], (DNV, DNV), "c1w_s")
            c1b_s = load(c1b_d[:], (DNV, 1), FP, "c1b_s")
            c2w_s = loadbf(c2w_d[:], (DNV, DNV), "c2w_s")
            c2b_s = load(c2b_d[:], (DNV, 1), FP, "c2b_s")
            mw_s = load(mw_d[:], (DNV, DNV), FP, "mw_s")
            mb_s = load(mb_d[:], (DNV, 1), FP, "mb_s")
            lw_s = load(lw_d[:], (DNV, DNV), FP, "lw_s")
            lb_s = load(lb_d[:], (DNV, 1), FP, "lb_s")

            wv_s, ww_s, we_s, wc_s, ub_s = [], [], [], [], []
            wih_s, whh_s, bih_s, bhh_s = [], [], [], []
            for k in range(NROUNDS):
                wv_s.append(loadbf(uw_d[k, 0:DNV, :], (DNV, DNV), f"wv{k}"))
                ww_s.append(loadbf(uw_d[k, DNV:2 * DNV, :], (DNV, DNV),
                                   f"ww{k}"))
                we_s.append(loadbf(uw_d[k, 2 * DNV:3 * DNV, :], (DNV, DNV),
                                   f"we{k}"))
                wc_s.append(load(uw_d[k, 3 * DNV:3 * DNV + NCOND, :],
                                 (NCOND, DNV), FP, f"wc{k}"))
                ub_s.append(load(ub_d[k], (DNV, 1), FP, f"ub{k}"))
                wih_s.append(loadbf(wih_d[k], (DNV, 3 * DNV), f"wih{k}"))
                whh_s.append(loadbf(whh_d[k], (DNV, 3 * DNV), f"whh{k}"))
                bih_s.append(load(bih_d[k], (DNV, 3), FP, f"bih{k}"))
                bhh_s.append(load(bhh_d[k], (DNV, 3), FP, f"bhh{k}"))

            ident = cst.tile([P, P], BF, name="ident")
            make_identity(nc, ident[:])

            SBC0 = SBW * P
            nsb0 = -(-nsh_pad // SBC0)
            hA = [cst.tile([P, min(SBC0, nsh_pad - s * SBC0)], BF,
                           name=f"hA{s}") for s in range(nsb0)]
            hB = [cst.tile([P, min(SBC0, nsh_pad - s * SBC0)], BF,
                           name=f"hB{s}") for s in range(nsb0)]
            eat = cst.tile([P, nsh_pad], BF, name="eat")

            ch1, ch2 = pl.ch1, pl.ch2
            hshA = [dram.tile([ch1, DNV], BF, name=f"hshA{k}")
                    for k in range(1, NROUNDS)]
            hshB = [dram.tile([ch2, DNV], BF, name=f"hshB{k}")
                    for k in range(1, NROUNDS)]
            htabA = [dram.tile([ncores * ch1, DNV], BF, name=f"htabA{k}",
                               addr_space="Shared")
                     for k in range(1, NROUNDS)]
            htabB = [dram.tile([ncores * ch2, DNV], BF, name=f"htabB{k}",
                               addr_space="Shared")
                     for k in range(1, NROUNDS)]
            rs_in = dram.tile([P, 1], FP, name="rs_in")
            rs_out = dram.tile([P, 1], FP, name="rs_out", addr_space="Shared")

            SBC = SBW * P

            def sb_cols(sb):
                lo = sb * SBC
                return lo, min(lo + SBC, nsh)

            def store_block(hbuf, k, b):
                lo = b * P
                wb = min(P, nsh - lo)
                sb0, off = b // SBW, (b % SBW) * P
                tp = gpsum.tile([P, P], BF, name=f"tp{k}_{b}", tag="g")
                nc.tensor.transpose(out=tp[:],
                                    in_=hbuf[sb0][:, off:off + P],
                                    identity=ident[:])
                hnode = work.tile([P, P], BF, name=f"hn{k}_{b}",
                                  tag="hnode")
                nc.scalar.copy(out=hnode[:wb, :], in_=tp[:wb, :])
                if lo < ch1:
                    dst = hshA[k - 1][lo:lo + wb, :]
                else:
                    dst = hshB[k - 1][lo - ch1:lo - ch1 + wb, :]
                nc.sync.dma_start(out=dst, in_=hnode[:wb, :])

            # ------------- h0 = node_feats @ W_node_emb (feature-major) ----
            for sb in range(nsb):
                lo, hi = sb_cols(sb)
                w = hi - lo
                nft = dpool.tile([NA_IN, SBC], FP, name=f"nft{sb}", tag="nft")
                nc.sync.dma_start(out=nft[:, :w], in_=nfT_d[:, lo:hi])
                ps = mpsum.tile([P, SBC], FP, name=f"h0ps{sb}", tag="m")
                nc.tensor.matmul(out=ps[:, :w], lhsT=wne_s[:],
                                 rhs=nft[:, :w], start=True, stop=True)
                nc.vector.tensor_copy(out=hA[sb][:, :w], in_=ps[:, :w])

            acc = cst.tile([P, 1], FP, name="acc")
            nc.vector.memset(acc[:], 0.0)

            # ------------- message-passing rounds --------------------------
            ag_insts = []
            for k in range(NROUNDS):
                ag_deps = list(ag_insts)
                ag_insts = []
                ndep = 0
                hcur = hA if k % 2 == 0 else hB
                hnxt = hB if k % 2 == 0 else hA

                cps = gpsum.tile([P, 1], FP, name=f"cps{k}", tag="g")
                nc.tensor.matmul(out=cps[:], lhsT=wc_s[k][:], rhs=condT_s[:],
                                 start=True, stop=True)
                ccol = work.tile([P, 1], FP, name=f"ccol{k}", tag="small")
                nc.scalar.activation(out=ccol[:], in_=cps[:], func=AF.Identity,
                                     bias=ub_s[k][:])
                bcmb = work.tile([P, 2], FP, name=f"bcmb{k}", tag="small2")
                nc.vector.tensor_tensor(out=bcmb[:], in0=bih_s[k][:, 0:2],
                                        in1=bhh_s[k][:, 0:2], op=OP.add)

                gi_of = {}
                gcur = 0
                for sb in range(nsb):
                    lo, hi = sb_cols(sb)
                    w = hi - lo
                    t_lo, t_hi = pl.sb_span[sb]
                    blks = list(range(sb * SBW, min((sb + 1) * SBW, nblk)))
                    if k > 0:
                        sps = spsum.tile([P, SBC], FP, name=f"sps{k}_{sb}",
                                         tag="s")
                    if k == 0:
                        ebp = spsum.tile([NC2, SBC], FP, name=f"ebp{sb}",
                                         tag="s")
                        eft_max = max(b - a for a, b in pl.sb_span)
                        eft = epool.tile(
                            [P, (t_hi - t_lo) * NC2], BF, name=f"eft{sb}",
                            tag="eft", padded_shape=[P, eft_max * NC2])
                        nc.gpsimd.dma_start(
                            out=eft[:],
                            in_=ef_d[:, t_lo * NC2:t_hi * NC2])

                    # gathers: one per (half, <=gmax tiles) run; one batched
                    # is_equal builds the whole group's selection matrices
                    tilemap = {}
                    done = t_lo
                    while done < t_hi:
                        h_, glo, ghi = pl.groups[gcur]
                        gcur += 1
                        assert glo == done
                        ng = ghi - glo
                        gbuf = None
                        if k > 0:
                            gbuf = gpool.tile([P, ng, DNV], BF,
                                              name=f"g{k}_{gcur}", tag="gbuf",
                                              padded_shape=[P, pl.gmax, DNV])
                            src = (htabA[k - 1].opt() if h_ == 0
                                   else htabB[k - 1].opt())
                            gin = nc.gpsimd.dma_gather(
                                out_ap=gbuf[:],
                                in_ap=src,
                                idxs_ap=idx_s[:, glo * 8:ghi * 8],
                                num_idxs=ng * P,
                                num_idxs_reg=nreg(ng * P),
                                elem_size=DNV,
                                queue_num=gcur % 4,
                            )
                            if ndep < 4 and ag_deps:
                                for ag in ag_deps:
                                    tile.add_dep_helper(
                                        gin.ins, ag.ins, sync=False,
                                        reason="gather after AG trigger")
                                ndep += 1
                        selg = selp.tile([P, ng, P], BF, name=f"sl{k}_{gcur}",
                                         tag="sel",
                                         padded_shape=[P, pl.gmax, P])
                        nc.vector.tensor_tensor(
                            out=selg[:],
                            in0=dst_s[:, glo:ghi, None].to_broadcast(
                                [P, ng, P]),
                            in1=iota_s[:, None, :].to_broadcast([P, ng, P]),
                            op=OP.is_equal)
                        for j in range(ng):
                            tilemap[glo + j] = (gbuf, selg, j)
                        done = ghi

                    # selection matmuls, block-major so that each block's
                    # PSUM accumulation group opens and closes contiguously
                    for b in blks:
                        bc = (b - sb * SBW) * P
                        for hh in (0, 1):
                            r_lo, r_hi = pl.trange[(b, hh)]
                            for t in range(r_lo, r_hi):
                                _b, _h, st, sp = pl.tiles[t]
                                gbuf, selg, j = tilemap[t]
                                if k > 0:
                                    nc.tensor.matmul(out=sps[:, bc:bc + P],
                                                     lhsT=gbuf[:, j, :],
                                                     rhs=selg[:, j, :],
                                                     start=st, stop=sp)
                                else:
                                    o = (t - t_lo) * NC2
                                    nc.tensor.matmul(
                                        out=ebp[:, bc:bc + P],
                                        lhsT=eft[:, o:o + NC2],
                                        rhs=selg[:, j, :], start=st, stop=sp)

                    degt = dpool.tile([P, SBC], BF, name=f"dg{k}_{sb}",
                                      tag="degt")
                    nc.gpsimd.dma_start(out=degt[:, :w], in_=degb_d[:, lo:hi])

                    # m = (deg*h)@Wv + S@Ww + EA@We + deg (x) cond_row
                    s_sb = work.tile([P, SBC], BF, name=f"ssb{k}_{sb}",
                                     tag="ssb")
                    if k == 0:
                        ebs = work.tile([NC2, SBC], BF, name=f"ebs{sb}",
                                        tag="ebs")
                        nc.vector.tensor_copy(out=ebs[:, :w], in_=ebp[:, :w])
                        eap = gpsum.tile([P, SBC], FP, name=f"eap{sb}",
                                         tag="g")
                        nc.tensor.matmul(out=eap[:, :w], lhsT=wee_s[:],
                                         rhs=ebs[0:NB_IN, :w], start=True,
                                         stop=True)
                        nc.scalar.copy(out=eat[:, lo:hi], in_=eap[:, :w])
                        s0p = gpsum.tile([P, SBC], FP, name=f"s0p{sb}",
                                         tag="g")
                        nc.tensor.matmul(out=s0p[:, :w],
                                         lhsT=wnebf_s[32:NC2, :],
                                         rhs=ebs[32:NC2, :w], start=True,
                                         stop=True)
                        nc.scalar.copy(out=s_sb[:, :w], in_=s0p[:, :w])
                    else:
                        nc.scalar.copy(out=s_sb[:, :w], in_=sps[:, :w])
                    hd = work.tile([P, SBC], BF, name=f"hd{k}_{sb}", tag="hd")
                    nc.vector.tensor_tensor(out=hd[:, :w], in0=hcur[sb][:, :w],
                                            in1=degt[:, :w], op=OP.mult)
                    mps = mpsum.tile([P, SBC], FP, name=f"mps{k}_{sb}",
                                     tag="m")
                    nc.tensor.matmul(out=mps[:, :w], lhsT=wv_s[k][:],
                                     rhs=hd[:, :w], start=True, stop=False)
                    nc.tensor.matmul(out=mps[:, :w], lhsT=ww_s[k][:],
                                     rhs=s_sb[:, :w], start=False, stop=False)
                    nc.tensor.matmul(out=mps[:, :w], lhsT=we_s[k][:],
                                     rhs=eat[:, lo:hi], start=False,
                                     stop=True)
                    dcnd = work.tile([P, SBC], BF, name=f"dc{k}_{sb}",
                                     tag="dcnd")
                    nc.vector.tensor_scalar(out=dcnd[:, :w],
                                            in0=degt[:, :w],
                                            scalar1=ccol[:], scalar2=None,
                                            op0=OP.mult)
                    m_sb = work.tile([P, SBC], BF, name=f"m{k}_{sb}",
                                     tag="msb")
                    nc.vector.tensor_tensor(out=m_sb[:, :w], in0=mps[:, :w],
                                            in1=dcnd[:, :w], op=OP.add)

                    # GRU gates (biases fused into activations)
                    grp = gpsum.tile([P, SBC], FP, name=f"gr{k}_{sb}",
                                     tag="g")
                    nc.tensor.matmul(out=grp[:, :w], lhsT=wih_s[k][:, 0:DNV],
                                     rhs=m_sb[:, :w], start=True, stop=False)
                    nc.tensor.matmul(out=grp[:, :w], lhsT=whh_s[k][:, 0:DNV],
                                     rhs=hcur[sb][:, :w], start=False,
                                     stop=True)
                    r_t = work.tile([P, SBC], BF, name=f"r{k}_{sb}", tag="rt")
                    nc.scalar.activation(out=r_t[:, :w], in_=grp[:, :w],
                                         func=AF.Sigmoid, bias=bcmb[:, 0:1])
                    gzp = gpsum.tile([P, SBC], FP, name=f"gz{k}_{sb}",
                                     tag="g")
                    nc.tensor.matmul(out=gzp[:, :w],
                                     lhsT=wih_s[k][:, DNV:2 * DNV],
                                     rhs=m_sb[:, :w], start=True, stop=False)
                    nc.tensor.matmul(out=gzp[:, :w],
                                     lhsT=whh_s[k][:, DNV:2 * DNV],
                                     rhs=hcur[sb][:, :w], start=False,
                                     stop=True)
                    z_t = work.tile([P, SBC], BF, name=f"z{k}_{sb}", tag="zt")
                    nc.scalar.activation(out=z_t[:, :w], in_=gzp[:, :w],
                                         func=AF.Sigmoid, bias=bcmb[:, 1:2])
                    hnp = gpsum.tile([P, SBC], FP, name=f"hp{k}_{sb}",
                                     tag="g")
                    nc.tensor.matmul(out=hnp[:, :w],
                                     lhsT=whh_s[k][:, 2 * DNV:3 * DNV],
                                     rhs=hcur[sb][:, :w], start=True,
                                     stop=True)
                    nnp = gpsum.tile([P, SBC], FP, name=f"np{k}_{sb}",
                                     tag="g")
                    nc.tensor.matmul(out=nnp[:, :w],
                                     lhsT=wih_s[k][:, 2 * DNV:3 * DNV],
                                     rhs=m_sb[:, :w], start=True, stop=True)
                    t1 = work.tile([P, SBC], BF, name=f"t1{k}_{sb}",
                                   tag="ssb2")
                    nc.scalar.activation(out=t1[:, :w], in_=hnp[:, :w],
                                         func=AF.Identity,
                                         bias=bhh_s[k][:, 2:3])
                    nc.vector.tensor_tensor(out=t1[:, :w], in0=r_t[:, :w],
                                            in1=t1[:, :w], op=OP.mult)
                    nc.vector.tensor_tensor(out=t1[:, :w], in0=nnp[:, :w],
                                            in1=t1[:, :w], op=OP.add)
                    n_t = work.tile([P, SBC], BF, name=f"n{k}_{sb}",
                                    tag="msb2")
                    nc.scalar.activation(out=n_t[:, :w], in_=t1[:, :w],
                                         func=AF.Tanh, bias=bih_s[k][:, 2:3])
                    d_t = work.tile([P, SBC], BF, name=f"dd{k}_{sb}",
                                    tag="hd2")
                    nc.vector.tensor_tensor(out=d_t[:, :w],
                                            in0=hcur[sb][:, :w],
                                            in1=n_t[:, :w], op=OP.subtract)
                    nc.vector.tensor_tensor(out=d_t[:, :w], in0=z_t[:, :w],
                                            in1=d_t[:, :w], op=OP.mult)
                    nc.vector.tensor_tensor(out=hnxt[sb][:, :w],
                                            in0=n_t[:, :w], in1=d_t[:, :w],
                                            op=OP.add)
                    if k == NROUNDS - 1:
                        g1 = gpsum.tile([P, SBC], FP, name=f"g1{sb}",
                                        tag="g")
                        nc.tensor.matmul(out=g1[:, :w], lhsT=c1w_s[:],
                                         rhs=hnxt[sb][:, :w], start=True,
                                         stop=True)
                        a1 = work.tile([P, SBC], FP, name=f"a1{sb}",
                                       tag="rt")
                        nc.scalar.activation(out=a1[:, :w], in_=g1[:, :w],
                                             func=AF.Identity, bias=c1b_s[:])
                        g2 = gpsum.tile([P, SBC], FP, name=f"g2{sb}",
                                        tag="g")
                        nc.tensor.matmul(out=g2[:, :w], lhsT=c2w_s[:],
                                         rhs=hnxt[sb][:, :w], start=True,
                                         stop=True)
                        a2 = work.tile([P, SBC], FP, name=f"a2{sb}",
                                       tag="zt")
                        nc.scalar.activation(out=a2[:, :w], in_=g2[:, :w],
                                             func=AF.Sigmoid, bias=c2b_s[:])
                        nc.vector.tensor_tensor(out=a1[:, :w],
                                                in0=a1[:, :w],
                                                in1=a2[:, :w], op=OP.mult)
                        part = work.tile([P, 1], FP, name=f"pa{sb}",
                                         tag="part")
                        nc.vector.reduce_sum(out=part[:], in_=a1[:, :w],
                                             axis=mybir.AxisListType.X)
                        nc.vector.tensor_tensor(out=acc[:], in0=acc[:],
                                                in1=part[:], op=OP.add)
                    if k + 1 < NROUNDS:
                        for b in blks:
                            store_block(hnxt, k + 1, b)
                            if b == ch1 // P - 1:
                                ag_insts.append(nc.gpsimd.collective_compute(
                                    "AllGather", OP.bypass,
                                    replica_groups=rg,
                                    ins=[hshA[k].opt()],
                                    outs=[htabA[k].opt()]))

                if k + 1 < NROUNDS:
                    ag_insts.append(nc.gpsimd.collective_compute(
                        "AllGather", OP.bypass, replica_groups=rg,
                        ins=[hshB[k].opt()], outs=[htabB[k].opt()]))

            # (readout is interleaved into round 2's super-block loop)

            nc.sync.dma_start(out=rs_in[:], in_=acc[:])
            nc.gpsimd.collective_compute(
                "AllReduce", OP.add, replica_groups=rg,
                ins=[rs_in.opt()], outs=[rs_out.opt()])
            tot = cst.tile([P, 1], FP, name="tot")
            nc.sync.dma_start(out=tot[:], in_=rs_out[:])
            enc = cst.tile([P, 1], FP, name="enc")
            nc.vector.tensor_scalar(out=enc[:], in0=tot[:], scalar1=inv_n,
                                    scalar2=None, op0=OP.mult)

            mup = gpsum.tile([P, 1], FP, name="mup", tag="g")
            nc.tensor.matmul(out=mup[:], lhsT=mw_s[:], rhs=enc[:],
                             start=True, stop=True)
            mu = cst.tile([P, 1], FP, name="mu")
            nc.scalar.activation(out=mu[:], in_=mup[:], func=AF.Identity,
                                 bias=mb_s[:])
            lvp = gpsum.tile([P, 1], FP, name="lvp", tag="g")
            nc.tensor.matmul(out=lvp[:], lhsT=lw_s[:], rhs=enc[:],
                             start=True, stop=True)
            lv = cst.tile([P, 1], FP, name="lv")
            nc.scalar.activation(out=lv[:], in_=lvp[:], func=AF.Identity,
                                 bias=lb_s[:])
            hlb = cst.tile([P, 1], FP, name="hlb")
            nc.vector.tensor_scalar(out=hlb[:], in0=lb_s[:], scalar1=0.5,
                                    scalar2=None, op0=OP.mult)
            ex = cst.tile([P, 1], FP, name="ex")
            nc.scalar.activation(out=ex[:], in_=lvp[:], func=AF.Exp,
                                 bias=hlb[:], scale=0.5)
            lat = cst.tile([P, 1], FP, name="lat")
            nc.vector.tensor_tensor(out=lat[:], in0=epsT_s[:], in1=ex[:],
                                    op=OP.mult)
            nc.vector.tensor_tensor(out=lat[:], in0=mu[:], in1=lat[:],
                                    op=OP.add)

            nc.sync.dma_start(out=out3[:, 0:1], in_=lat[:])
            nc.sync.dma_start(out=out3[:, 1:2], in_=mu[:])
            nc.sync.dma_start(out=out3[:, 2:3], in_=lv[:])

    nc.compile()
    return nc


def make_inputs(pl, inputs, core):
    """Per-core input map (host reshapes/permutes only)."""
    w = {}
    w["gidx"] = np.ascontiguousarray(pl.gidx[core])
    w["dstloc"] = np.ascontiguousarray(pl.dstloc[core])
    w["ef"] = np.ascontiguousarray(pl.ef[core])
    w["degb"] = np.ascontiguousarray(pl.degb[core])
    w["nfT"] = np.ascontiguousarray(pl.nfT[core])
    w["iota"] = np.broadcast_to(np.arange(P, dtype=np.float32), (P, P)).copy()
    w["wne"] = np.ascontiguousarray(inputs["W_node_emb"], np.float32)
    w["wee"] = np.ascontiguousarray(inputs["W_edge_emb"], np.float32)
    w["uw"] = np.ascontiguousarray(inputs["enc_U_W"], np.float32)
    w["ub"] = np.ascontiguousarray(inputs["enc_U_b"], np.float32)[..., None]
    w["wih"] = np.ascontiguousarray(inputs["enc_gru_Wih"], np.float32)
    w["whh"] = np.ascontiguousarray(inputs["enc_gru_Whh"], np.float32)
    w["bih"] = np.ascontiguousarray(
        inputs["enc_gru_bih"].reshape(NROUNDS, 3, DNV).transpose(0, 2, 1),
        np.float32)
    w["bhh"] = np.ascontiguousarray(
        inputs["enc_gru_bhh"].reshape(NROUNDS, 3, DNV).transpose(0, 2, 1),
        np.float32)
    w["condT"] = np.ascontiguousarray(inputs["condition"].T, np.float32)
    w["epsT"] = np.ascontiguousarray(inputs["eps"].T, np.float32)
    w["c1w"] = np.ascontiguousarray(inputs["cal_enc1_W"], np.float32)
    w["c1b"] = np.ascontiguousarray(inputs["cal_enc1_b"], np.float32)[:, None]
    w["c2w"] = np.ascontiguousarray(inputs["cal_enc2_W"], np.float32)
    w["c2b"] = np.ascontiguousarray(inputs["cal_enc2_b"], np.float32)[:, None]
    w["mw"] = np.ascontiguousarray(inputs["mean_W"], np.float32)
    w["mb"] = np.ascontiguousarray(inputs["mean_b"], np.float32)[:, None]
    w["lw"] = np.ascontiguousarray(inputs["logvar_W"], np.float32)
    w["lb"] = np.ascontiguousarray(inputs["logvar_b"], np.float32)[:, None]
    return w


def prepare(inputs, n=N_FULL, ncores=NCORES):
    inputs = {k: np.asarray(v) for k, v in inputs.items()}
    pl = build_plan(inputs["edge_src"].astype(np.int64),
                    inputs["edge_dst"].astype(np.int64),
                    inputs["edge_feats"], inputs["node_feats"], n, ncores)
    nc = build_program(pl, ncores)
    in_maps = [make_inputs(pl, inputs, c) for c in range(ncores)]
    return nc, in_maps


def run(inputs, n=N_FULL, ncores=NCORES, trace=False):
    from concourse.bass_utils import run_bass_kernel_spmd

    nc, in_maps = prepare(inputs, n, ncores)
    res = run_bass_kernel_spmd(nc, in_maps, list(range(ncores)), trace=trace)
    o = res.results[0]["out3"]          # [128, 3]: latent, mu, logvar columns
    out = np.ascontiguousarray(o.T).reshape(1, 3 * DNV).astype(np.float32)
    return out, res


def kernel(**inputs):
    out, _ = run(inputs)
    return out


# revision 48
# speedup vs baseline: 1.0363x; 1.0363x over previous
"""Trainium2 Bass kernel for 3-round GNN message passing + GRU encoder (GGM).

Math (per round k):
    msg_e = h[dst_e] @ Wv + h[src_e] @ Ww + e_e @ We + cond_row_k      [E, 128]
    m_v   = sum_{e: dst_e = v} msg_e                                  [N, 128]
    h_v   = GRU_k(m_v, h_v)
Since segment-sum is linear this is restructured as
    m_v = (deg_v * h_v) @ Wv + S_v @ Ww + EA_v @ We + deg_v * cond_row_k
with S_v = sum over in-edges of h[src] (the only gather-heavy term) and
EA_v = sum over in-edges of e = (segment_sum(edge_feats, dst)) @ W_edge_emb
(round-independent; accumulated on device during round 0's selection matmuls).

Sharding: nodes are split into 8 contiguous shards; each core owns the
in-edges of its shard (dst-sorted, in 128-edge tiles).  The gather table
(full h, node-major) is replicated in HBM and rebuilt each round with an
8-core AllGather.  Per-tile segment sums: a selection matrix Sel[e, d] =
(dstloc[e] == d) built by one is_equal op turns the segment-sum into a
PSUM-accumulated matmul  S^T[:, blk] += G_tile^T-style  (out = lhsT.T @ rhs
with lhsT = gathered h rows, rhs = Sel).  All node compute is feature-major
([128 feat] x nodes) so weights are natural lhsT operands and biases fuse
into ScalarE activations.  Readout: local free-dim reduction + AllReduce.
"""

import os
import sys

import numpy as np

for _p in ("/opt/trn_rl_repo",):
    if os.path.isdir(_p) and _p not in sys.path:
        sys.path.insert(0, _p)

import concourse.bacc as bacc
import concourse.bass as bass
import concourse.bass_isa as bass_isa
import concourse.mybir as mybir
import concourse.tile as tile
from concourse import tile_sem_assignment as _tsa
from concourse.masks import make_identity


def _install_queue_aware_dma_lanes():
    """Make Tile's SWDGE completion-sem lane choice queue-consistent.

    Tile round-robins Pool-engine DMA instructions over the 8 DMASW sem
    lanes with no knowledge of the SWDGE queue they run on, but a sem lane
    must only ever be updated from one SWDGE queue (ucode reclaim tracks
    per-queue).  Derive the lane from the instruction's queue instead:
    queue q uses lanes {2q, 2q+1}.
    """
    if getattr(_tsa.TileClockTick, "_ggm_patched", False):
        return
    orig = _tsa.TileClockTick._assign_tick

    def patched(self, inst):
        if (isinstance(inst, _tsa.DMAInst)
                and inst.engine == mybir.EngineType.Pool
                and not isinstance(inst, bass_isa.UserSyncedRemoteDMADescs)):
            q = int(getattr(inst, "queue_num", 0) or 0)
            cnts = getattr(self, "_ggm_qcnt", None)
            if cnts is None:
                cnts = self._ggm_qcnt = [0, 0, 0, 0]
            self.next_sw_dma_idx = 2 * q + (cnts[q] & 1)
            cnts[q] += 1
        return orig(self, inst)

    _tsa.TileClockTick._assign_tick = patched
    _tsa.TileClockTick._ggm_patched = True

FP = mybir.dt.float32
BF = mybir.dt.bfloat16
I16 = mybir.dt.int16

P = 128
DNV = 128
NA_IN = 14
NB_IN = 11
NCOND = 4
NROUNDS = 3

N_FULL = 50000
NCORES = 8

SBW = 4          # dst blocks per super-block (matmul free dim up to 512)
GCAP = 8         # max edge tiles per dma_gather: 1024 idxs = 1024 descs,
                 # the SWDGE ring capacity (16KB / 16B) — more wedges the HW
DST_SENTINEL = 300.0


# --------------------------------------------------------------------------
# host-side planning (integer ops / permutations only)
# --------------------------------------------------------------------------

class Plan:
    pass


def build_plan(edge_src, edge_dst, edge_feats, node_feats, n, ncores):
    """Shard + dst-sort edges; pad to a tile structure common to all cores."""
    pl = Plan()
    assert n % ncores == 0
    nsh = n // ncores
    assert nsh % P != 0 or True
    nblk = -(-nsh // P)
    nsb = -(-nblk // SBW)
    half = (n + 1) // 2
    pl.n, pl.nsh, pl.nblk, pl.nsb, pl.half = n, nsh, nblk, nsb, half
    pl.nsh_pad = nblk * P

    assert nblk >= 2
    ch1 = (nblk // 2) * P          # chunk-1 nodes per core (block-aligned)
    ch2 = nsh - ch1
    assert ncores * ch1 < 32768 and ncores * ch2 < 32768
    pl.ch1, pl.ch2 = ch1, ch2
    core_of = edge_dst // nsh
    per_core = []
    cnt = np.zeros((ncores, nblk, 2), np.int64)
    for c in range(ncores):
        sel = np.nonzero(core_of == c)[0]
        d = edge_dst[sel] - c * nsh
        s = edge_src[sel]
        s_core = s // nsh
        s_loc = s - s_core * nsh
        hlf = (s_loc >= ch1).astype(np.int64)
        blk = d // P
        order = np.lexsort((d, blk, hlf, blk // SBW))
        gval = np.where(hlf == 0, s_core * ch1 + s_loc,
                        s_core * ch2 + (s_loc - ch1)).astype(np.int16)
        sel, d, gval, hlf, blk = (sel[order], d[order], gval[order],
                                  hlf[order], blk[order])
        per_core.append((sel, d, gval, hlf, blk))
        np.add.at(cnt[c], (blk, hlf), 1)

    # common per-(block,half) tile counts: max over cores, at least 1
    ntile = np.maximum(1, -(-cnt.max(axis=0) // P))   # [nblk, 2]
    pl.ntile = ntile

    # ordered tile list (per sblock: half A blocks asc, then half B) and
    # gather groups (runs of tiles with one src half, <= gmax tiles)
    tiles = []    # (blk, half, start_flag, stop_flag)
    runs = []     # (half, tile_lo, tile_hi) one per (sblock, half)
    sb_span = []  # (tile_lo, tile_hi) per sblock
    trange = {}   # (blk, half) -> (tile_lo, tile_hi)
    for sb in range(nsb):
        blks = range(sb * SBW, min((sb + 1) * SBW, nblk))
        t_sb0 = len(tiles)
        for h in (0, 1):
            glo = len(tiles)
            for b in blks:
                trange[(b, h)] = (len(tiles), len(tiles) + ntile[b, h])
                for t in range(ntile[b, h]):
                    tiles.append((b, h, h == 0 and t == 0,
                                  h == 1 and t == ntile[b, h] - 1))
            runs.append((h, glo, len(tiles)))
        sb_span.append((t_sb0, len(tiles)))
    T = len(tiles)
    gmax = min(GCAP, max(hi - lo for _h, lo, hi in runs))
    groups = []   # (half, tile_lo, tile_hi) gather instructions
    for h, glo, ghi in runs:
        lo = glo
        while lo < ghi:
            hi = min(lo + gmax, ghi)
            groups.append((h, lo, hi))
            lo = hi
    pl.tiles, pl.groups, pl.T, pl.sb_span = tiles, groups, T, sb_span
    pl.trange, pl.gmax = trange, gmax
    # all of one super-block's gather groups are live at once (each block's
    # PSUM accumulation reads from both halves' groups) — size the pool for
    # that plus one slot of next-super-block prefetch
    per_sb = []
    gi = 0
    for lo_sb, hi_sb in sb_span:
        cnt = 0
        while gi < len(groups) and groups[gi][1] < hi_sb:
            cnt += 1
            gi += 1
        per_sb.append(cnt)
    pl.gbufs = max(per_sb) + 1

    # base slot of each (blk, half) run in the padded edge stream
    base = {}
    slot = 0
    for (b, h, _st, _sp) in tiles:
        if (b, h) not in base:
            base[(b, h)] = slot
        slot += P

    NC2 = 32 + NA_IN   # ef at cols 0..11, nf at cols 32..46 (matmul base-partition rule)
    pl.gidx = np.zeros((ncores, P, T * 8), np.int16)
    pl.dstloc = np.full((ncores, P, T), DST_SENTINEL, np.float32)
    pl.ef = np.zeros((ncores, P, T * NC2), np.float32)
    pl.degb = np.zeros((ncores, P, pl.nsh_pad), np.float32)
    pl.nfT = np.zeros((ncores, NA_IN, pl.nsh_pad), np.float32)

    deg_full = np.bincount(edge_dst, minlength=n).astype(np.float32)
    nfT_full = np.ascontiguousarray(node_feats.T.astype(np.float32))

    for c in range(ncores):
        sel, d, gval, hlf, blk = per_core[c]
        ne = len(sel)
        posn = np.zeros(ne, np.int64)
        if ne:
            key = blk * 2 + hlf
            bounds = np.nonzero(np.diff(key) != 0)[0] + 1
            starts = np.concatenate(([0], bounds))
            ends = np.concatenate((bounds, [ne]))
            for a, b_ in zip(starts, ends):
                posn[a:b_] = np.arange(b_ - a)
        slots = (np.array([base[(bb, hh)] for bb, hh in zip(blk, hlf)],
                          np.int64) + posn)
        gidx = gval
        col = (slots // 128) * 8 + (slots % 128) // 16
        row = slots % 16
        for r in range(8):
            pl.gidx[c, row + 16 * r, col] = gidx
        pl.dstloc[c, slots % P, slots // P] = (d - blk * P).astype(np.float32)
        ef_rows = edge_feats[sel].astype(np.float32)
        nf_rows = node_feats[edge_src[sel]].astype(np.float32)  # h0[src]
        tidx, pp = slots // P, slots % P
        for j in range(NB_IN):
            pl.ef[c, pp, tidx * NC2 + j] = ef_rows[:, j]
        for j in range(NA_IN):
            pl.ef[c, pp, tidx * NC2 + 32 + j] = nf_rows[:, j]
        pl.degb[c, :, :nsh] = np.broadcast_to(
            deg_full[c * nsh:(c + 1) * nsh], (P, nsh))
        pl.nfT[c, :, :nsh] = nfT_full[:, c * nsh:(c + 1) * nsh]

    return pl


# --------------------------------------------------------------------------
# device program
# --------------------------------------------------------------------------

def build_program(pl, ncores):
    _install_queue_aware_dma_lanes()
    nc = bacc.Bacc("TRN2", target_bir_lowering=False, debug=False,
                   num_devices=ncores, num_swdge_queues=4)
    n, nsh, nblk, nsb, T = pl.n, pl.nsh, pl.nblk, pl.nsb, pl.T
    half = pl.half
    nsh_pad = pl.nsh_pad
    inv_n = 1.0 / float(n)
    AF = mybir.ActivationFunctionType
    OP = mybir.AluOpType

    def din(name, shape, dtype=FP):
        return nc.dram_tensor(name, list(shape), dtype, kind="ExternalInput")

    gidx_d = din("gidx", (P, T * 8), I16)
    dstloc_d = din("dstloc", (P, T))
    NC2 = 32 + NA_IN
    ef_d = din("ef", (P, T * NC2))
    degb_d = din("degb", (P, nsh_pad))
    nfT_d = din("nfT", (NA_IN, nsh_pad))
    iota_d = din("iota", (P, P))
    wne_d = din("wne", (NA_IN, DNV))
    wee_d = din("wee", (NB_IN, DNV))
    uw_d = din("uw", (NROUNDS, 2 * DNV + DNV + NCOND, DNV))
    ub_d = din("ub", (NROUNDS, DNV, 1))
    wih_d = din("wih", (NROUNDS, DNV, 3 * DNV))
    whh_d = din("whh", (NROUNDS, DNV, 3 * DNV))
    bih_d = din("bih", (NROUNDS, DNV, 3))
    bhh_d = din("bhh", (NROUNDS, DNV, 3))
    condT_d = din("condT", (NCOND, 1))
    epsT_d = din("epsT", (DNV, 1))
    c1w_d = din("c1w", (DNV, DNV))
    c1b_d = din("c1b", (DNV, 1))
    c2w_d = din("c2w", (DNV, DNV))
    c2b_d = din("c2b", (DNV, 1))
    mw_d = din("mw", (DNV, DNV))
    mb_d = din("mb", (DNV, 1))
    lw_d = din("lw", (DNV, DNV))
    lb_d = din("lb", (DNV, 1))

    out3 = nc.dram_tensor("out3", [P, 3], FP, kind="ExternalOutput")

    rg = [list(range(ncores))]

    with tile.TileContext(nc) as tc:
        with (
            tc.tile_pool(name="cst", bufs=1) as cst,
            tc.tile_pool(name="dram", bufs=1, space="DRAM") as dram,
            tc.tile_pool(name="gpool", bufs=pl.gbufs) as gpool,
            tc.tile_pool(name="selp", bufs=pl.gbufs) as selp,
            tc.tile_pool(name="epool", bufs=2) as epool,
            tc.tile_pool(name="dpool", bufs=2) as dpool,
            tc.tile_pool(name="work", bufs=2) as work,
            tc.tile_pool(name="spsum", bufs=2, space="PSUM") as spsum,
            tc.tile_pool(name="mpsum", bufs=2, space="PSUM") as mpsum,
            tc.tile_pool(name="gpsum", bufs=4, space="PSUM") as gpsum,
        ):
            _regs = {}

            def nreg(v):
                if v not in _regs:
                    _regs[v] = nc.gpsimd.to_reg(v)
                return _regs[v]

            def load(dr_ap, shape, dtype=FP, name="w"):
                t = cst.tile(list(shape), dtype, name=name)
                nc.sync.dma_start(out=t[:], in_=dr_ap)
                return t

            def loadbf(dr_ap, shape, name="w"):
                t = cst.tile(list(shape), BF, name=name)
                nc.gpsimd.dma_start(out=t[:], in_=dr_ap)  # f32 -> bf16 cast
                return t

            idx_s = load(gidx_d[:], (P, T * 8), I16, "idx_s")
            dst_s = cst.tile([P, T], BF, name="dst_s")
            nc.gpsimd.dma_start(out=dst_s[:], in_=dstloc_d[:])  # f32->bf16
            iota_s = cst.tile([P, P], BF, name="iota_s")
            nc.gpsimd.dma_start(out=iota_s[:], in_=iota_d[:])
            wne_s = load(wne_d[:], (NA_IN, DNV), FP, "wne_s")
            wnebf_s = cst.tile([32 + NA_IN, DNV], BF, name="wnebf_s")
            nc.gpsimd.dma_start(out=wnebf_s[32:32 + NA_IN, :], in_=wne_d[:])
            wee_s = loadbf(wee_d[:], (NB_IN, DNV), "wee_s")
            condT_s = load(condT_d[:], (NCOND, 1), FP, "condT_s")
            epsT_s = load(epsT_d[:], (DNV, 1), FP, "epsT_s")
            c1w_s = loadbf(c1w_d[:], (DNV, DNV), "c1w_s")
            c1b_s = load(c1b_d[:], (DNV, 1), FP, "c1b_s")
            c2w_s = loadbf(c2w_d[:], (DNV, DNV), "c2w_s")
            c2b_s = load(c2b_d[:], (DNV, 1), FP, "c2b_s")
            mw_s = load(mw_d[:], (DNV, DNV), FP, "mw_s")
            mb_s = load(mb_d[:], (DNV, 1), FP, "mb_s")
            lw_s = load(lw_d[:], (DNV, DNV), FP, "lw_s")
            lb_s = load(lb_d[:], (DNV, 1), FP, "lb_s")

            wv_s, ww_s, we_s, wc_s, ub_s = [], [], [], [], []
            wih_s, whh_s, bih_s, bhh_s = [], [], [], []
            for k in range(NROUNDS):
                wv_s.append(loadbf(uw_d[k, 0:DNV, :], (DNV, DNV), f"wv{k}"))
                ww_s.append(loadbf(uw_d[k, DNV:2 * DNV, :], (DNV, DNV),
                                   f"ww{k}"))
                we_s.append(loadbf(uw_d[k, 2 * DNV:3 * DNV, :], (DNV, DNV),
                                   f"we{k}"))
                wc_s.append(load(uw_d[k, 3 * DNV:3 * DNV + NCOND, :],
                                 (NCOND, DNV), FP, f"wc{k}"))
                ub_s.append(load(ub_d[k], (DNV, 1), FP, f"ub{k}"))
                wih_s.append(loadbf(wih_d[k], (DNV, 3 * DNV), f"wih{k}"))
                whh_s.append(loadbf(whh_d[k], (DNV, 3 * DNV), f"whh{k}"))
                bih_s.append(load(bih_d[k], (DNV, 3), FP, f"bih{k}"))
                bhh_s.append(load(bhh_d[k], (DNV, 3), FP, f"bhh{k}"))

            ident = cst.tile([P, P], BF, name="ident")
            make_identity(nc, ident[:])

            SBC0 = SBW * P
            nsb0 = -(-nsh_pad // SBC0)
            hA = [cst.tile([P, min(SBC0, nsh_pad - s * SBC0)], BF,
                           name=f"hA{s}") for s in range(nsb0)]
            hB = [cst.tile([P, min(SBC0, nsh_pad - s * SBC0)], BF,
                           name=f"hB{s}") for s in range(nsb0)]
            eat = cst.tile([P, nsh_pad], BF, name="eat")

            ch1, ch2 = pl.ch1, pl.ch2
            hshA = [dram.tile([ch1, DNV], BF, name=f"hshA{k}")
                    for k in range(1, NROUNDS)]
            hshB = [dram.tile([ch2, DNV], BF, name=f"hshB{k}")
                    for k in range(1, NROUNDS)]
            htabA = [dram.tile([ncores * ch1, DNV], BF, name=f"htabA{k}",
                               addr_space="Shared")
                     for k in range(1, NROUNDS)]
            htabB = [dram.tile([ncores * ch2, DNV], BF, name=f"htabB{k}",
                               addr_space="Shared")
                     for k in range(1, NROUNDS)]
            rs_in = dram.tile([P, 1], FP, name="rs_in")
            rs_out = dram.tile([P, 1], FP, name="rs_out", addr_space="Shared")

            SBC = SBW * P

            def sb_cols(sb):
                lo = sb * SBC
                return lo, min(lo + SBC, nsh)

            def store_block(hbuf, k, b):
                lo = b * P
                wb = min(P, nsh - lo)
                sb0, off = b // SBW, (b % SBW) * P
                tp = gpsum.tile([P, P], BF, name=f"tp{k}_{b}", tag="g")
                nc.tensor.transpose(out=tp[:],
                                    in_=hbuf[sb0][:, off:off + P],
                                    identity=ident[:])
                hnode = work.tile([P, P], BF, name=f"hn{k}_{b}",
                                  tag="hnode")
                nc.scalar.copy(out=hnode[:wb, :], in_=tp[:wb, :])
                if lo < ch1:
                    dst = hshA[k - 1][lo:lo + wb, :]
                else:
                    dst = hshB[k - 1][lo - ch1:lo - ch1 + wb, :]
                nc.sync.dma_start(out=dst, in_=hnode[:wb, :])

            # ------------- h0 = node_feats @ W_node_emb (feature-major) ----
            for sb in range(nsb):
                lo, hi = sb_cols(sb)
                w = hi - lo
                nft = dpool.tile([NA_IN, SBC], FP, name=f"nft{sb}", tag="nft")
                nc.sync.dma_start(out=nft[:, :w], in_=nfT_d[:, lo:hi])
                ps = mpsum.tile([P, SBC], FP, name=f"h0ps{sb}", tag="m")
                nc.tensor.matmul(out=ps[:, :w], lhsT=wne_s[:],
                                 rhs=nft[:, :w], start=True, stop=True)
                nc.vector.tensor_copy(out=hA[sb][:, :w], in_=ps[:, :w])

            acc = cst.tile([P, 1], FP, name="acc")
            nc.vector.memset(acc[:], 0.0)

            # ------------- message-passing rounds --------------------------
            ag_insts = []
            for k in range(NROUNDS):
                ag_deps = list(ag_insts)
                ag_insts = []
                ndep = 0
                hcur = hA if k % 2 == 0 else hB
                hnxt = hB if k % 2 == 0 else hA

                cps = gpsum.tile([P, 1], FP, name=f"cps{k}", tag="g")
                nc.tensor.matmul(out=cps[:], lhsT=wc_s[k][:], rhs=condT_s[:],
                                 start=True, stop=True)
                ccol = work.tile([P, 1], FP, name=f"ccol{k}", tag="small")
                nc.scalar.activation(out=ccol[:], in_=cps[:], func=AF.Identity,
                                     bias=ub_s[k][:])
                bcmb = work.tile([P, 2], FP, name=f"bcmb{k}", tag="small2")
                nc.vector.tensor_tensor(out=bcmb[:], in0=bih_s[k][:, 0:2],
                                        in1=bhh_s[k][:, 0:2], op=OP.add)

                gi_of = {}
                gcur = 0
                for sb in range(nsb):
                    lo, hi = sb_cols(sb)
                    w = hi - lo
                    t_lo, t_hi = pl.sb_span[sb]
                    blks = list(range(sb * SBW, min((sb + 1) * SBW, nblk)))
                    if k > 0:
                        sps = spsum.tile([P, SBC], FP, name=f"sps{k}_{sb}",
                                         tag="s")
                    if k == 0:
                        ebp = spsum.tile([NC2, SBC], FP, name=f"ebp{sb}",
                                         tag="s")
                        eft_max = max(b - a for a, b in pl.sb_span)
                        eft = epool.tile(
                            [P, (t_hi - t_lo) * NC2], BF, name=f"eft{sb}",
                            tag="eft", padded_shape=[P, eft_max * NC2])
                        nc.gpsimd.dma_start(
                            out=eft[:],
                            in_=ef_d[:, t_lo * NC2:t_hi * NC2])

                    # gathers: one per (half, <=gmax tiles) run; one batched
                    # is_equal builds the whole group's selection matrices
                    tilemap = {}
                    done = t_lo
                    while done < t_hi:
                        h_, glo, ghi = pl.groups[gcur]
                        gcur += 1
                        assert glo == done
                        ng = ghi - glo
                        gbuf = None
                        if k > 0:
                            gbuf = gpool.tile([P, ng, DNV], BF,
                                              name=f"g{k}_{gcur}", tag="gbuf",
                                              padded_shape=[P, pl.gmax, DNV])
                            src = (htabA[k - 1].opt() if h_ == 0
                                   else htabB[k - 1].opt())
                            gin = nc.gpsimd.dma_gather(
                                out_ap=gbuf[:],
                                in_ap=src,
                                idxs_ap=idx_s[:, glo * 8:ghi * 8],
                                num_idxs=ng * P,
                                num_idxs_reg=nreg(ng * P),
                                elem_size=DNV,
                                queue_num=gcur % 4,
                            )
                            if ndep < 4 and ag_deps:
                                for ag in ag_deps:
                                    tile.add_dep_helper(
                                        gin.ins, ag.ins, sync=False,
                                        reason="gather after AG trigger")
                                ndep += 1
                        selg = selp.tile([P, ng, P], BF, name=f"sl{k}_{gcur}",
                                         tag="sel",
                                         padded_shape=[P, pl.gmax, P])
                        nc.vector.tensor_tensor(
                            out=selg[:],
                            in0=dst_s[:, glo:ghi, None].to_broadcast(
                                [P, ng, P]),
                            in1=iota_s[:, None, :].to_broadcast([P, ng, P]),
                            op=OP.is_equal)
                        for j in range(ng):
                            tilemap[glo + j] = (gbuf, selg, j)
                        done = ghi

                    # selection matmuls, block-major so that each block's
                    # PSUM accumulation group opens and closes contiguously
                    for b in blks:
                        bc = (b - sb * SBW) * P
                        for hh in (0, 1):
                            r_lo, r_hi = pl.trange[(b, hh)]
                            for t in range(r_lo, r_hi):
                                _b, _h, st, sp = pl.tiles[t]
                                gbuf, selg, j = tilemap[t]
                                if k > 0:
                                    nc.tensor.matmul(out=sps[:, bc:bc + P],
                                                     lhsT=gbuf[:, j, :],
                                                     rhs=selg[:, j, :],
                                                     start=st, stop=sp)
                                else:
                                    o = (t - t_lo) * NC2
                                    nc.tensor.matmul(
                                        out=ebp[:, bc:bc + P],
                                        lhsT=eft[:, o:o + NC2],
                                        rhs=selg[:, j, :], start=st, stop=sp)

                    degt = dpool.tile([P, SBC], BF, name=f"dg{k}_{sb}",
                                      tag="degt")
                    nc.gpsimd.dma_start(out=degt[:, :w], in_=degb_d[:, lo:hi])

                    # m = (deg*h)@Wv + S@Ww + EA@We + deg (x) cond_row
                    s_sb = work.tile([P, SBC], BF, name=f"ssb{k}_{sb}",
                                     tag="ssb")
                    if k == 0:
                        ebs = work.tile([NC2, SBC], BF, name=f"ebs{sb}",
                                        tag="ebs")
                        nc.vector.tensor_copy(out=ebs[:, :w], in_=ebp[:, :w])
                        eap = gpsum.tile([P, SBC], FP, name=f"eap{sb}",
                                         tag="g")
                        nc.tensor.matmul(out=eap[:, :w], lhsT=wee_s[:],
                                         rhs=ebs[0:NB_IN, :w], start=True,
                                         stop=True)
                        nc.scalar.copy(out=eat[:, lo:hi], in_=eap[:, :w])
                        s0p = gpsum.tile([P, SBC], FP, name=f"s0p{sb}",
                                         tag="g")
                        nc.tensor.matmul(out=s0p[:, :w],
                                         lhsT=wnebf_s[32:NC2, :],
                                         rhs=ebs[32:NC2, :w], start=True,
                                         stop=True)
                        nc.scalar.copy(out=s_sb[:, :w], in_=s0p[:, :w])
                    else:
                        nc.scalar.copy(out=s_sb[:, :w], in_=sps[:, :w])
                    hd = work.tile([P, SBC], BF, name=f"hd{k}_{sb}", tag="hd")
                    nc.vector.tensor_tensor(out=hd[:, :w], in0=hcur[sb][:, :w],
                                            in1=degt[:, :w], op=OP.mult)
                    mps = mpsum.tile([P, SBC], FP, name=f"mps{k}_{sb}",
                                     tag="m")
                    nc.tensor.matmul(out=mps[:, :w], lhsT=wv_s[k][:],
                                     rhs=hd[:, :w], start=True, stop=False)
                    nc.tensor.matmul(out=mps[:, :w], lhsT=ww_s[k][:],
                                     rhs=s_sb[:, :w], start=False, stop=False)
                    nc.tensor.matmul(out=mps[:, :w], lhsT=we_s[k][:],
                                     rhs=eat[:, lo:hi], start=False,
                                     stop=True)
                    dcnd = work.tile([P, SBC], BF, name=f"dc{k}_{sb}",
                                     tag="dcnd")
                    nc.vector.tensor_scalar(out=dcnd[:, :w],
                                            in0=degt[:, :w],
                                            scalar1=ccol[:], scalar2=None,
                                            op0=OP.mult)
                    m_sb = work.tile([P, SBC], BF, name=f"m{k}_{sb}",
                                     tag="msb")
                    nc.vector.tensor_tensor(out=m_sb[:, :w], in0=mps[:, :w],
                                            in1=dcnd[:, :w], op=OP.add)

                    # GRU gates (biases fused into activations)
                    grp = gpsum.tile([P, SBC], FP, name=f"gr{k}_{sb}",
                                     tag="g")
                    nc.tensor.matmul(out=grp[:, :w], lhsT=wih_s[k][:, 0:DNV],
                                     rhs=m_sb[:, :w], start=True, stop=False)
                    nc.tensor.matmul(out=grp[:, :w], lhsT=whh_s[k][:, 0:DNV],
                                     rhs=hcur[sb][:, :w], start=False,
                                     stop=True)
                    r_t = work.tile([P, SBC], BF, name=f"r{k}_{sb}", tag="rt")
                    nc.scalar.activation(out=r_t[:, :w], in_=grp[:, :w],
                                         func=AF.Sigmoid, bias=bcmb[:, 0:1])
                    gzp = gpsum.tile([P, SBC], FP, name=f"gz{k}_{sb}",
                                     tag="g")
                    nc.tensor.matmul(out=gzp[:, :w],
                                     lhsT=wih_s[k][:, DNV:2 * DNV],
                                     rhs=m_sb[:, :w], start=True, stop=False)
                    nc.tensor.matmul(out=gzp[:, :w],
                                     lhsT=whh_s[k][:, DNV:2 * DNV],
                                     rhs=hcur[sb][:, :w], start=False,
                                     stop=True)
                    z_t = work.tile([P, SBC], BF, name=f"z{k}_{sb}", tag="zt")
                    nc.scalar.activation(out=z_t[:, :w], in_=gzp[:, :w],
                                         func=AF.Sigmoid, bias=bcmb[:, 1:2])
                    hnp = gpsum.tile([P, SBC], FP, name=f"hp{k}_{sb}",
                                     tag="g")
                    nc.tensor.matmul(out=hnp[:, :w],
                                     lhsT=whh_s[k][:, 2 * DNV:3 * DNV],
                                     rhs=hcur[sb][:, :w], start=True,
                                     stop=True)
                    nnp = gpsum.tile([P, SBC], FP, name=f"np{k}_{sb}",
                                     tag="g")
                    nc.tensor.matmul(out=nnp[:, :w],
                                     lhsT=wih_s[k][:, 2 * DNV:3 * DNV],
                                     rhs=m_sb[:, :w], start=True, stop=True)
                    t1 = work.tile([P, SBC], BF, name=f"t1{k}_{sb}",
                                   tag="ssb2")
                    nc.scalar.activation(out=t1[:, :w], in_=hnp[:, :w],
                                         func=AF.Identity,
                                         bias=bhh_s[k][:, 2:3])
                    nc.vector.tensor_tensor(out=t1[:, :w], in0=r_t[:, :w],
                                            in1=t1[:, :w], op=OP.mult)
                    nc.vector.tensor_tensor(out=t1[:, :w], in0=nnp[:, :w],
                                            in1=t1[:, :w], op=OP.add)
                    n_t = work.tile([P, SBC], BF, name=f"n{k}_{sb}",
                                    tag="msb2")
                    nc.scalar.activation(out=n_t[:, :w], in_=t1[:, :w],
                                         func=AF.Tanh, bias=bih_s[k][:, 2:3])
                    d_t = work.tile([P, SBC], BF, name=f"dd{k}_{sb}",
                                    tag="hd2")
                    nc.vector.tensor_tensor(out=d_t[:, :w],
                                            in0=hcur[sb][:, :w],
                                            in1=n_t[:, :w], op=OP.subtract)
                    nc.vector.tensor_tensor(out=d_t[:, :w], in0=z_t[:, :w],
                                            in1=d_t[:, :w], op=OP.mult)
                    nc.vector.tensor_tensor(out=hnxt[sb][:, :w],
                                            in0=n_t[:, :w], in1=d_t[:, :w],
                                            op=OP.add)
                    if k + 1 < NROUNDS:
                        for b in blks:
                            store_block(hnxt, k + 1, b)
                            if b == ch1 // P - 1:
                                ag_insts.append(nc.gpsimd.collective_compute(
                                    "AllGather", OP.bypass,
                                    replica_groups=rg,
                                    ins=[hshA[k].opt()],
                                    outs=[htabA[k].opt()]))

                if k + 1 < NROUNDS:
                    ag_insts.append(nc.gpsimd.collective_compute(
                        "AllGather", OP.bypass, replica_groups=rg,
                        ins=[hshB[k].opt()], outs=[htabB[k].opt()]))

            # ------------- gated readout + reparameterize ------------------
            hfin = hB if NROUNDS % 2 == 1 else hA
            for sb in range(nsb):
                lo, hi = sb_cols(sb)
                w = hi - lo
                g1 = gpsum.tile([P, SBC], FP, name=f"g1{sb}", tag="g")
                nc.tensor.matmul(out=g1[:, :w], lhsT=c1w_s[:],
                                 rhs=hfin[sb][:, :w], start=True, stop=True)
                a1 = work.tile([P, SBC], FP, name=f"a1{sb}", tag="rt")
                nc.scalar.activation(out=a1[:, :w], in_=g1[:, :w],
                                     func=AF.Identity, bias=c1b_s[:])
                g2 = gpsum.tile([P, SBC], FP, name=f"g2{sb}", tag="g")
                nc.tensor.matmul(out=g2[:, :w], lhsT=c2w_s[:],
                                 rhs=hfin[sb][:, :w], start=True, stop=True)
                a2 = work.tile([P, SBC], FP, name=f"a2{sb}", tag="zt")
                nc.scalar.activation(out=a2[:, :w], in_=g2[:, :w],
                                     func=AF.Sigmoid, bias=c2b_s[:])
                nc.vector.tensor_tensor(out=a1[:, :w], in0=a1[:, :w],
                                        in1=a2[:, :w], op=OP.mult)
                part = work.tile([P, 1], FP, name=f"pa{sb}", tag="part")
                nc.vector.reduce_sum(out=part[:], in_=a1[:, :w],
                                     axis=mybir.AxisListType.X)
                nc.vector.tensor_tensor(out=acc[:], in0=acc[:], in1=part[:],
                                        op=OP.add)

            nc.sync.dma_start(out=rs_in[:], in_=acc[:])
            nc.gpsimd.collective_compute(
                "AllReduce", OP.add, replica_groups=rg,
                ins=[rs_in.opt()], outs=[rs_out.opt()])
            tot = cst.tile([P, 1], FP, name="tot")
            nc.sync.dma_start(out=tot[:], in_=rs_out[:])
            enc = cst.tile([P, 1], FP, name="enc")
            nc.vector.tensor_scalar(out=enc[:], in0=tot[:], scalar1=inv_n,
                                    scalar2=None, op0=OP.mult)

            mup = gpsum.tile([P, 1], FP, name="mup", tag="g")
            nc.tensor.matmul(out=mup[:], lhsT=mw_s[:], rhs=enc[:],
                             start=True, stop=True)
            mu = cst.tile([P, 1], FP, name="mu")
            nc.scalar.activation(out=mu[:], in_=mup[:], func=AF.Identity,
                                 bias=mb_s[:])
            lvp = gpsum.tile([P, 1], FP, name="lvp", tag="g")
            nc.tensor.matmul(out=lvp[:], lhsT=lw_s[:], rhs=enc[:],
                             start=True, stop=True)
            lv = cst.tile([P, 1], FP, name="lv")
            nc.scalar.activation(out=lv[:], in_=lvp[:], func=AF.Identity,
                                 bias=lb_s[:])
            hlb = cst.tile([P, 1], FP, name="hlb")
            nc.vector.tensor_scalar(out=hlb[:], in0=lb_s[:], scalar1=0.5,
                                    scalar2=None, op0=OP.mult)
            ex = cst.tile([P, 1], FP, name="ex")
            nc.scalar.activation(out=ex[:], in_=lvp[:], func=AF.Exp,
                                 bias=hlb[:], scale=0.5)
            lat = cst.tile([P, 1], FP, name="lat")
            nc.vector.tensor_tensor(out=lat[:], in0=epsT_s[:], in1=ex[:],
                                    op=OP.mult)
            nc.vector.tensor_tensor(out=lat[:], in0=mu[:], in1=lat[:],
                                    op=OP.add)

            nc.sync.dma_start(out=out3[:, 0:1], in_=lat[:])
            nc.sync.dma_start(out=out3[:, 1:2], in_=mu[:])
            nc.sync.dma_start(out=out3[:, 2:3], in_=lv[:])

    nc.compile()
    return nc


def make_inputs(pl, inputs, core):
    """Per-core input map (host reshapes/permutes only)."""
    w = {}
    w["gidx"] = np.ascontiguousarray(pl.gidx[core])
    w["dstloc"] = np.ascontiguousarray(pl.dstloc[core])
    w["ef"] = np.ascontiguousarray(pl.ef[core])
    w["degb"] = np.ascontiguousarray(pl.degb[core])
    w["nfT"] = np.ascontiguousarray(pl.nfT[core])
    w["iota"] = np.broadcast_to(np.arange(P, dtype=np.float32), (P, P)).copy()
    w["wne"] = np.ascontiguousarray(inputs["W_node_emb"], np.float32)
    w["wee"] = np.ascontiguousarray(inputs["W_edge_emb"], np.float32)
    w["uw"] = np.ascontiguousarray(inputs["enc_U_W"], np.float32)
    w["ub"] = np.ascontiguousarray(inputs["enc_U_b"], np.float32)[..., None]
    w["wih"] = np.ascontiguousarray(inputs["enc_gru_Wih"], np.float32)
    w["whh"] = np.ascontiguousarray(inputs["enc_gru_Whh"], np.float32)
    w["bih"] = np.ascontiguousarray(
        inputs["enc_gru_bih"].reshape(NROUNDS, 3, DNV).transpose(0, 2, 1),
        np.float32)
    w["bhh"] = np.ascontiguousarray(
        inputs["enc_gru_bhh"].reshape(NROUNDS, 3, DNV).transpose(0, 2, 1),
        np.float32)
    w["condT"] = np.ascontiguousarray(inputs["condition"].T, np.float32)
    w["epsT"] = np.ascontiguousarray(inputs["eps"].T, np.float32)
    w["c1w"] = np.ascontiguousarray(inputs["cal_enc1_W"], np.float32)
    w["c1b"] = np.ascontiguousarray(inputs["cal_enc1_b"], np.float32)[:, None]
    w["c2w"] = np.ascontiguousarray(inputs["cal_enc2_W"], np.float32)
    w["c2b"] = np.ascontiguousarray(inputs["cal_enc2_b"], np.float32)[:, None]
    w["mw"] = np.ascontiguousarray(inputs["mean_W"], np.float32)
    w["mb"] = np.ascontiguousarray(inputs["mean_b"], np.float32)[:, None]
    w["lw"] = np.ascontiguousarray(inputs["logvar_W"], np.float32)
    w["lb"] = np.ascontiguousarray(inputs["logvar_b"], np.float32)[:, None]
    return w


def prepare(inputs, n=N_FULL, ncores=NCORES):
    inputs = {k: np.asarray(v) for k, v in inputs.items()}
    pl = build_plan(inputs["edge_src"].astype(np.int64),
                    inputs["edge_dst"].astype(np.int64),
                    inputs["edge_feats"], inputs["node_feats"], n, ncores)
    nc = build_program(pl, ncores)
    in_maps = [make_inputs(pl, inputs, c) for c in range(ncores)]
    return nc, in_maps


def run(inputs, n=N_FULL, ncores=NCORES, trace=False):
    from concourse.bass_utils import run_bass_kernel_spmd

    nc, in_maps = prepare(inputs, n, ncores)
    res = run_bass_kernel_spmd(nc, in_maps, list(range(ncores)), trace=trace)
    o = res.results[0]["out3"]          # [128, 3]: latent, mu, logvar columns
    out = np.ascontiguousarray(o.T).reshape(1, 3 * DNV).astype(np.float32)
    return out, res


def kernel(**inputs):
    out, _ = run(inputs)
    return out
